# revision 1
# baseline (speedup 1.0000x reference)
"""GraphWaveNet block kernel for 8 Trainium2 NeuronCores.

Math (reference reduced):
  A = gcn_norm adjacency [N,N] (dense, built on host from edge lists)
  The res_w branch is dead code (never feeds the output).
  A commutes with every channel-mixing conv, and the start conv is a
  rank-1 channel lift, so the whole filter/gate path collapses to:
    fg[b,o,m,t] = v0[o]*xA[b,m,t] + v1[o]*xA[b,m,t+1] + rowsumA[m]*bfg[o] + gcn_b[o]
    g [b,o,n,t] = p0[o]*x[b,t,n] + p1[o]*x[b,t+1,n] + bg[o]
    hg = tanh(fg)*sigmoid(g)
    out = mean_t relu(end1 @ relu(skip @ hg + skip_b) + end1_b) -> end2
  with xA[b] = A @ x[b].T and v0/v1/p0/p1/bfg/bg tiny host-folded vectors.

Sharding: data-parallel over batch, 1 batch element per core (B=8).
All inputs arrive as ONE packed [128, F] DRAM tensor so every on-chip
consumer waits on a single DMA semaphore (PE matmuls have 1 wait slot).
"""

import numpy as np

import concourse.bass as bass
from concourse import bacc
from concourse import mybir
from concourse.bass_utils import run_bass_kernel_spmd
from concourse.tile import TileContext

FP = mybir.dt.float32
FPR = mybir.dt.float32r
BF = mybir.dt.bfloat16

B, T, N, E = 8, 32, 512, 8192
TO = T - 1          # output time steps
RC = DC = 64
SC, EC, P = 256, 512, 12
NCORES = 8
NT = N // 128       # node tiles

# packed-constant layout: name -> free-dim width of the [128, w] segment
_SEGS = [
    ("xT", NT * T),
    ("at", NT * N),
    ("cb", NT * RC),
    ("v0", RC), ("v1", RC), ("p0", RC), ("p1", RC), ("bg", RC),
    ("skt", SC),
    ("skb", SC // 128),
    ("e1t", (SC // 128) * EC),
    ("e1b", EC // 128),
    ("e2t", (EC // 128) * P),
    ("e2b", 1),
    ("ident", 128),
]
_OFF = {}
_F = 0
for _nm, _w in _SEGS:
    _OFF[_nm] = _F
    _F += _w


def _gcn_adj(edge_index, edge_weight, n):
    ei = np.asarray(edge_index)
    ew = np.asarray(edge_weight, dtype=np.float64)
    ar = np.arange(n)
    row = np.concatenate([ei[0], ar])
    col = np.concatenate([ei[1], ar])
    w = np.concatenate([ew, np.ones(n)])
    deg = np.zeros(n)
    np.add.at(deg, col, w)
    dis = np.where(deg > 0, 1.0 / np.sqrt(np.maximum(deg, 1e-300)), 0.0)
    norm = dis[row] * w * dis[col]
    A = np.zeros((n, n))
    np.add.at(A, (col, row), norm)
    return A  # A[tgt, src]


def _build_nc():
    nc = bacc.Bacc()
    d_c = nc.declare_dram_parameter("C", [128, _F], FP, isOutput=False)
    d_out = nc.declare_dram_parameter("out", [P, N], FP, isOutput=True)

    AluOp = mybir.AluOpType
    Act = mybir.ActivationFunctionType

    def seg(ct, nm, w):
        return ct[:, _OFF[nm]:_OFF[nm] + w]

    with TileContext(nc) as tc:
        with (
            tc.tile_pool(name="consts", bufs=1) as consts,
            tc.tile_pool(name="ew", bufs=2) as ew,
            tc.tile_pool(name="hg", bufs=2) as hgp,
            tc.tile_pool(name="hgcm", bufs=3) as hgcmp,
            tc.tile_pool(name="acts", bufs=3) as actsp,
            tc.tile_pool(name="accum", bufs=1) as accum,
            tc.tile_pool(name="ps_xa", bufs=1, space="PSUM") as ps_xa,  # also used for e2
            tc.tile_pool(name="ps_tr", bufs=2, space="PSUM") as ps_tr,
            tc.tile_pool(name="ps_sk", bufs=2, space="PSUM") as ps_sk,
            tc.tile_pool(name="ps_e1", bufs=3, space="PSUM") as ps_e1,
        ):
            ct = consts.tile([128, _F], FP)
            nc.sync.dma_start(out=ct, in_=d_c[:])

            xT = seg(ct, "xT", NT * T).rearrange("p (i t) -> p i t", i=NT)
            at = seg(ct, "at", NT * N).rearrange("p (i n) -> p i n", i=NT)
            cb = seg(ct, "cb", NT * RC).rearrange("p (i o) -> p i o", i=NT)
            v0 = seg(ct, "v0", RC)
            v1 = seg(ct, "v1", RC)
            p0 = seg(ct, "p0", RC)
            p1 = seg(ct, "p1", RC)
            bg = seg(ct, "bg", RC)
            skb = seg(ct, "skb", SC // 128)
            e1b = seg(ct, "e1b", EC // 128)
            e2b = seg(ct, "e2b", 1)
            ident = seg(ct, "ident", 128)

            # FP32r weight copies (engine write rounds to FP32r)
            skt = consts.tile([2 * RC, SC], FPR)
            nc.scalar.copy(out=skt[:], in_=seg(ct, "skt", SC)[:2 * RC])
            e1t = consts.tile([128, SC // 128, EC], FPR)
            nc.scalar.copy(
                out=e1t[:],
                in_=seg(ct, "e1t", (SC // 128) * EC)
                .rearrange("p (k m) -> p k m", k=SC // 128),
            )
            e2t = consts.tile([128, EC // 128, P], FPR)
            nc.scalar.copy(
                out=e2t[:],
                in_=seg(ct, "e2t", (EC // 128) * P)
                .rearrange("p (k m) -> p k m", k=EC // 128),
            )

            identb = consts.tile([128, 128], BF)
            nc.scalar.copy(out=identb[:], in_=ident)

            # ---- phase A: xA[m, t] = sum_n A[m, n] x[t, n] ----
            xa = consts.tile([128, NT, T], FP)
            for i in range(NT):
                xps_full = ps_xa.tile([128, N], FP, tag="xps")
                xps = xps_full[:, :T]
                for kt in range(NT):
                    nc.tensor.matmul(
                        xps[:],
                        at[:, kt, i * 128:(i + 1) * 128],
                        xT[:, kt, :],
                        start=(kt == 0),
                        stop=(kt == NT - 1),
                    )
                nc.scalar.copy(out=xa[:, i, :], in_=xps[:])

            # ---- accumulators for mean over t of relu(end1) ----
            macc = []
            for mj in range(EC // 128):
                m = accum.tile([128, N], FPR, tag=f"macc{mj}")
                nc.vector.memset(m[:].bitcast(FP), 0.0)
                macc.append(m)

            # time chunks (output t = 0..TO-1), sized even for pairing
            chunks = [(0, 8), (8, 8), (16, 8), (24, 7)]

            for (t0, L) in chunks:
                # ---- phase B: hgT[m, t, o] node-major, per node-tile ----
                hgts = []
                for i in range(NT):
                    shp = [128, L, RC]
                    fga = ew.tile(shp, FP, tag=f"fga{i}")
                    fgb = ew.tile(shp, FP, tag=f"fgb{i}")
                    # fg = xa[:,t]*v0 + xa[:,t+1]*v1 + cb
                    nc.vector.tensor_tensor(
                        fga[:],
                        xa[:, i, t0:t0 + L, None].to_broadcast(shp),
                        v0[:, None, :].to_broadcast(shp),
                        AluOp.mult,
                    )
                    nc.vector.tensor_tensor(
                        fgb[:],
                        xa[:, i, t0 + 1:t0 + 1 + L, None].to_broadcast(shp),
                        v1[:, None, :].to_broadcast(shp),
                        AluOp.mult,
                    )
                    nc.gpsimd.tensor_tensor(fga[:], fga[:], fgb[:], AluOp.add)
                    nc.gpsimd.tensor_tensor(
                        fga[:], fga[:],
                        cb[:, i, None, :].to_broadcast(shp),
                        AluOp.add,
                    )
                    nc.scalar.activation(fga[:], fga[:], Act.Tanh)
                    # g = x[:,t]*p0 + x[:,t+1]*p1 + bg
                    ga = ew.tile(shp, FP, tag=f"ga{i}")
                    gb = ew.tile(shp, FP, tag=f"gb{i}")
                    nc.vector.tensor_tensor(
                        ga[:],
                        xT[:, i, t0:t0 + L, None].to_broadcast(shp),
                        p0[:, None, :].to_broadcast(shp),
                        AluOp.mult,
                    )
                    nc.vector.tensor_tensor(
                        gb[:],
                        xT[:, i, t0 + 1:t0 + 1 + L, None].to_broadcast(shp),
                        p1[:, None, :].to_broadcast(shp),
                        AluOp.mult,
                    )
                    nc.gpsimd.tensor_tensor(ga[:], ga[:], gb[:], AluOp.add)
                    nc.gpsimd.tensor_tensor(
                        ga[:], ga[:],
                        bg[:, None, :].to_broadcast(shp),
                        AluOp.add,
                    )
                    nc.scalar.activation(ga[:], ga[:], Act.Sigmoid)
                    hgt = hgp.tile(shp, BF, tag=f"hgt{i}")
                    nc.vector.tensor_tensor(hgt[:], fga[:], ga[:], AluOp.mult)
                    hgts.append(hgt)

                # ---- phase C: transpose pairs of t -> channel-major ----
                npairs = L // 2
                for j in range(npairs + (L % 2)):
                    tl = 2 if j < npairs else 1
                    tp = ps_tr.tile([128, NT, 128], BF, tag="trp")
                    for i in range(NT):
                        nc.tensor.transpose(
                            tp[:tl * 64, i, :],
                            hgts[i][:, 2 * j:2 * j + tl, :],
                            identb[:],
                        )
                    hgcm = hgcmp.tile([128, NT, 128], FPR, tag="hgcm")
                    nc.scalar.copy(out=hgcm[:tl * 64], in_=tp[:tl * 64])

                    for par in range(tl):
                        t = t0 + 2 * j + par
                        rhs = hgcm[par * 64:(par + 1) * 64]
                        # skip conv: [SC, N] = skip_w @ hg
                        rss = []
                        for sj in range(SC // 128):
                            skps = ps_sk.tile([128, N], FP, tag="skps")
                            nc.tensor.matmul(
                                skps[:],
                                skt[par * 64:(par + 1) * 64,
                                    sj * 128:(sj + 1) * 128],
                                rhs,
                                start=True,
                                stop=True,
                            )
                            rs = actsp.tile([128, N], FPR, tag=f"rs{sj}")
                            nc.scalar.activation(
                                rs[:], skps[:], Act.Relu,
                                bias=skb[:, sj:sj + 1], scale=1.0,
                            )
                            rss.append(rs)
                        # end1: [EC, N], K = SC
                        for mj in range(EC // 128):
                            e1ps = ps_e1.tile([128, N], FP, tag="e1ps")
                            for kj in range(SC // 128):
                                nc.tensor.matmul(
                                    e1ps[:],
                                    e1t[:, kj, mj * 128:(mj + 1) * 128],
                                    rss[kj][:],
                                    start=(kj == 0),
                                    stop=(kj == SC // 128 - 1),
                                )
                            r1 = actsp.tile([128, N], FP, tag="r1")
                            nc.scalar.activation(
                                r1[:], e1ps[:], Act.Relu,
                                bias=e1b[:, mj:mj + 1], scale=1.0,
                            )
                            nc.vector.tensor_tensor(
                                macc[mj][:], macc[mj][:], r1[:], AluOp.add,
                            )

            # ---- end2 on time-mean ----
            e2ps_full = ps_xa.tile([128, N], FP, tag="xps")
            e2ps = e2ps_full[:P]
            for kj in range(EC // 128):
                nc.tensor.matmul(
                    e2ps[:],
                    e2t[:, kj, :],
                    macc[kj][:],
                    start=(kj == 0),
                    stop=(kj == EC // 128 - 1),
                )
            outsb = consts.tile([P, N], FP)
            nc.scalar.activation(
                outsb[:], e2ps[:], Act.Identity,
                bias=e2b[:P, 0:1], scale=1.0 / TO,
            )
            nc.sync.dma_start(out=d_out[:], in_=outsb[:])

    return nc


_NC_CACHE = {}


def _get_nc():
    if "nc" not in _NC_CACHE:
        nc = _build_nc()
        nc.finalize()
        _NC_CACHE["nc"] = nc
    return _NC_CACHE["nc"]


def kernel(x, edge_index, edge_weight, start_w, start_b, filt_w, filt_b,
           gate_w, gate_b, gcn_w, gcn_b, res_w, res_b, skip_w, skip_b,
           end1_w, end1_b, end2_w, end2_b, **_unused):
    x = np.asarray(x, dtype=np.float32)
    A = _gcn_adj(edge_index, edge_weight, N)          # float64 [tgt, src]
    rowsum = A.sum(axis=1)

    f64 = lambda a: np.asarray(a, dtype=np.float64)
    s = f64(start_w)[:, 0]
    sb = f64(start_b)
    fw, gw = f64(filt_w), f64(gate_w)
    gcn = f64(gcn_w)
    v0 = gcn @ (fw[:, :, 0] @ s)
    v1 = gcn @ (fw[:, :, 1] @ s)
    bfg = gcn @ ((fw[:, :, 0] + fw[:, :, 1]) @ sb + f64(filt_b))
    p0 = gw[:, :, 0] @ s
    p1 = gw[:, :, 1] @ s
    bgv = (gw[:, :, 0] + gw[:, :, 1]) @ sb + f64(gate_b)
    cbT = np.outer(rowsum, bfg) + f64(gcn_b)[None, :]  # [N, RC]

    def part(a, ktiles):  # [(ktiles*128), M] -> [128, ktiles*M]
        a = np.asarray(a)
        return a.reshape(ktiles, 128, -1).transpose(1, 0, 2).reshape(128, -1)

    AT = np.ascontiguousarray(A.T)                     # [src n, tgt m]

    pack = np.zeros((128, _F), dtype=np.float32)

    def put(nm, arr):
        w = dict(_SEGS)[nm]
        a = np.asarray(arr, dtype=np.float32)
        pack[:a.shape[0], _OFF[nm]:_OFF[nm] + w] = a

    put("at", part(AT, NT))
    put("cb", part(cbT, NT))
    put("v0", np.tile(v0, (128, 1)))
    put("v1", np.tile(v1, (128, 1)))
    put("p0", np.tile(p0, (128, 1)))
    put("p1", np.tile(p1, (128, 1)))
    put("bg", np.tile(bgv, (128, 1)))
    put("skt", np.tile(f64(skip_w).T, (2, 1)))
    put("skb", f64(skip_b).reshape(SC // 128, 128).T)
    put("e1t", part(f64(end1_w).T, SC // 128))
    put("e1b", f64(end1_b).reshape(EC // 128, 128).T)
    put("e2t", part(f64(end2_w).T, EC // 128))
    put("e2b", np.asarray(end2_b).reshape(P, 1))
    put("ident", np.eye(128))

    in_maps = []
    for b in range(B):
        pk = pack.copy()
        xTb = np.ascontiguousarray(x[b].T)             # [N, T]
        pk[:, _OFF["xT"]:_OFF["xT"] + NT * T] = part(xTb, NT)
        in_maps.append({"C": pk})

    nc = _get_nc()
    _NC_CACHE["in_maps"] = in_maps
    res = run_bass_kernel_spmd(nc, in_maps, list(range(NCORES)))
    out = np.stack([res.results[i]["out"] for i in range(B)])
    return out.astype(np.float32)                       # [B, P, N]



# revision 6
# speedup vs baseline: 1.7655x; 1.7655x over previous
"""GraphWaveNet block kernel for 8 Trainium2 NeuronCores (v2).

Math (reference reduced; res_w branch is dead code):
  A = gcn_norm adjacency [N,N]; xa[m,t] = sum_n A[m,n] x[t,n]
  fg[o,m,t] = v0[o] xa[m,t] + v1[o] xa[m,t+1] + rowsum[m] bfg[o] + gcn_b[o]
  g [o,n,t] = p0[o] x[t,n]  + p1[o] x[t+1,n]  + bg[o]
  hg = tanh(fg) * sigmoid(g)
  out = end2 @ mean_t relu(end1 @ relu(skip @ hg + skb) + e1b) / TO + e2b

Design vs v1 baseline:
  - fg AND g for one t come out of a single K=66 fp16 matmul, channel-major
    (contract over a stacked rhs R = [xaT(32); x(32); rowsum; ones]), so no
    PE transposes and no broadcast elementwise ops.
  - all big GEMMs run in fp16 (PE full rate, half the SBUF read power of
    fp32r -> HAM stays at 2.4 GHz), PSUM accumulation stays fp32.
  - loop software-skewed: fg at t, skip at t-1, end1 at t-2, so the PE
    queue never waits on Act/DVE within a step.
  - relu+bias split across Act (activation bias operand) and DVE
    (tensor_scalar add+max); time-mean via fp16 pair-adds on DVE and
    fp32 accumulation on the otherwise idle GpSimd.

Sharding: data-parallel over batch, 1 batch element per core (B=8).
"""

import numpy as np

import concourse.bass as bass  # noqa: F401  (engine types via bacc)
from concourse import bacc
from concourse import mybir
from concourse.bass_utils import run_bass_kernel_spmd
from concourse.tile import TileContext

FP = mybir.dt.float32
FPR = mybir.dt.float32r
F16 = mybir.dt.float16

B, T, N, E = 8, 32, 512, 8192
TO = T - 1          # output time steps
RC = DC = 64
SC, EC, P = 256, 512, 12
NCORES = 8
NT = N // 128       # node tiles
KR = 66             # stacked-rhs contraction: 32 xaT + 32 x + rowsum + ones

# fp16 packed-constant layout: name -> free-dim width of the [128, w] segment
_SEGS16 = [
    ("xT", NT * T),            # lhsT tiles for phase A  [128 n, (kt, t)]
    ("at", NT * N),            # A^T tiles (rhs phase A) [128 n, (kt, m)]
    ("xnr", N),                # rows 0-31 x natural; row 32 rowsum; row 33 ones
    ("wt", TO * 128),          # rows 0-65: per-t fg/g lhsT matrices
    ("skt", SC),               # rows 0-63: skip_w^T
    ("e1t", (SC // 128) * EC),  # end1_w^T tiles [128, (kj, m)]
]
_OFF16 = {}
_F16 = 0
for _nm, _w in _SEGS16:
    _OFF16[_nm] = _F16
    _F16 += _w

# fp32 packed-constant layout
_SEGS32 = [
    ("skb", SC // 128),
    ("e1b", EC // 128),
    ("e2t", (EC // 128) * P),
    ("e2b", 1),
]
_OFF32 = {}
_F32 = 0
for _nm, _w in _SEGS32:
    _OFF32[_nm] = _F32
    _F32 += _w


def _gcn_adj(edge_index, edge_weight, n):
    ei = np.asarray(edge_index)
    ew = np.asarray(edge_weight, dtype=np.float64)
    ar = np.arange(n)
    row = np.concatenate([ei[0], ar])
    col = np.concatenate([ei[1], ar])
    w = np.concatenate([ew, np.ones(n)])
    deg = np.zeros(n)
    np.add.at(deg, col, w)
    dis = np.where(deg > 0, 1.0 / np.sqrt(np.maximum(deg, 1e-300)), 0.0)
    norm = dis[row] * w * dis[col]
    A = np.zeros((n, n))
    np.add.at(A, (col, row), norm)
    return A  # A[tgt, src]


def _build_nc():
    nc = bacc.Bacc()
    d_h = nc.declare_dram_parameter("H", [128, _F16], F16, isOutput=False)
    d_f = nc.declare_dram_parameter("F", [128, _F32], FP, isOutput=False)
    d_out = nc.declare_dram_parameter("out", [P, N], FP, isOutput=True)

    AluOp = mybir.AluOpType
    Act = mybir.ActivationFunctionType

    o_xT = _OFF16["xT"]
    o_at = _OFF16["at"]
    o_xnr = _OFF16["xnr"]
    o_wt = _OFF16["wt"]
    o_skt = _OFF16["skt"]
    o_e1t = _OFF16["e1t"]
    o_skb = _OFF32["skb"]
    o_e1b = _OFF32["e1b"]
    o_e2t = _OFF32["e2t"]
    o_e2b = _OFF32["e2b"]

    with TileContext(nc) as tc:
        with (
            tc.tile_pool(name="consts", bufs=1) as consts,
            tc.tile_pool(name="work", bufs=2) as work,
            tc.tile_pool(name="accum", bufs=1) as accum,
            tc.tile_pool(name="ps", bufs=1, space="PSUM") as ps,
        ):
            ct = consts.tile([128, _F16], F16)
            # DMA 1: phase-A inputs (xT + at); DMA 2: everything else.
            nc.sync.dma_start(out=ct[:, :o_xnr], in_=d_h[:, :o_xnr])
            nc.sync.dma_start(out=ct[:, o_xnr:], in_=d_h[:, o_xnr:])
            cf = consts.tile([128, _F32], FP)
            nc.sync.dma_start(out=cf[:], in_=d_f[:])
            # stacked rhs R: rows 0-31 xaT (computed), 32-63 x, 64 rowsum,
            # 65 ones
            R = consts.tile([KR, N], F16)
            nc.sync.dma_start(out=R[32:KR], in_=d_h[0:34, o_xnr:o_xnr + N])

            # prime the one activation table set during the DMA wait
            prime = consts.tile([1, 1], FP)
            nc.vector.memset(prime[:], 0.0)
            nc.scalar.activation(prime[:], prime[:], Act.Sigmoid)

            # FP32r copy of end2 weights (engine write rounds to FP32r)
            e2t = consts.tile([128, (EC // 128) * P], FPR)
            nc.scalar.copy(
                out=e2t[:], in_=cf[:, o_e2t: o_e2t + (EC // 128) * P])

            # ---- phase A: xaT[t, m] = sum_n x[t, n] AT[n, m] ----
            xaps = ps.tile([32, N], FP, tag="pa", bufs=1)
            for kt in range(NT):
                nc.tensor.matmul(
                    xaps[:],
                    ct[:, o_xT + kt * T: o_xT + (kt + 1) * T],
                    ct[:, o_at + kt * N: o_at + (kt + 1) * N],
                    start=(kt == 0),
                    stop=(kt == NT - 1),
                )
            nc.scalar.copy(out=R[0:32], in_=xaps[:])

            # ---- time-mean accumulators ----
            maccs = []
            for mj in range(EC // 128):
                m = accum.tile([128, N], FPR, tag=f"macc{mj}")
                nc.gpsimd.memset(m[:].bitcast(FP), 0.0)
                maccs.append(m)

            hgs = {}
            rsss = {}
            r1s = {}

            # skewed main loop: fg at s, skip at s-1, end1 at s-2
            for s in range(TO + 2):
                if s < TO:
                    t = s
                    fgps = ps.tile([128, N], FP, tag="fg", bufs=2)
                    nc.tensor.matmul(
                        fgps[:],
                        ct[0:KR, o_wt + t * 128: o_wt + (t + 1) * 128],
                        R[:],
                        start=True,
                        stop=True,
                    )
                    tnh = work.tile([64, N], F16, tag="tnh", bufs=2)
                    sgm = work.tile([64, N], F16, tag="sgm", bufs=2)
                    nc.scalar.activation(tnh[:], fgps[0:64], Act.Tanh)
                    nc.scalar.activation(sgm[:], fgps[64:128], Act.Sigmoid)

                tk = s - 1
                if 0 <= tk < TO:
                    hg = hgs.pop(tk)
                    sks = []
                    for sj in range(SC // 128):
                        skps = ps.tile([128, N], FP, tag="sk", bufs=2)
                        nc.tensor.matmul(
                            skps[:],
                            ct[0:64, o_skt + sj * 128: o_skt + (sj + 1) * 128],
                            hg[:],
                            start=True,
                            stop=True,
                        )
                        sks.append(skps)
                    rss = []
                    for sj in range(SC // 128):
                        rs = work.tile([128, N], F16, tag=f"rss{sj}", bufs=3)
                        bias = cf[:, o_skb + sj: o_skb + sj + 1]
                        if sj == 0:
                            nc.scalar.activation(
                                rs[:], sks[sj][:], Act.Relu,
                                bias=bias, scale=1.0,
                            )
                        else:
                            nc.vector.tensor_scalar(
                                rs[:], sks[sj][:], bias, 0.0,
                                AluOp.add, AluOp.max,
                            )
                        rss.append(rs)
                    rsss[tk] = rss

                te = s - 2
                if 0 <= te < TO:
                    rss = rsss.pop(te)
                    e1list = []
                    for mj in range(EC // 128):
                        e1ps = ps.tile([128, N], FP, tag="e1", bufs=3)
                        for kj in range(SC // 128):
                            nc.tensor.matmul(
                                e1ps[:],
                                ct[:, o_e1t + kj * EC + mj * 128:
                                   o_e1t + kj * EC + (mj + 1) * 128],
                                rss[kj][:],
                                start=(kj == 0),
                                stop=(kj == SC // 128 - 1),
                            )
                        e1list.append(e1ps)
                    r1l = []
                    for mj in range(EC // 128):
                        r1 = work.tile([128, N], F16, tag=f"r1_{mj}", bufs=2)
                        bias = cf[:, o_e1b + mj: o_e1b + mj + 1]
                        if mj == 0:
                            nc.scalar.activation(
                                r1[:], e1list[mj][:], Act.Relu,
                                bias=bias, scale=1.0,
                            )
                        else:
                            nc.vector.tensor_scalar(
                                r1[:], e1list[mj][:], bias, 0.0,
                                AluOp.add, AluOp.max,
                            )
                        r1l.append(r1)
                    r1s[te] = r1l
                    if te % 2 == 1:
                        prev = r1s.pop(te - 1)
                        cur = r1s.pop(te)
                        for mj in range(EC // 128):
                            pr = work.tile([128, N], F16, tag=f"pr{mj}",
                                           bufs=2)
                            nc.vector.tensor_tensor(
                                pr[:], prev[mj][:], cur[mj][:], AluOp.add,
                            )
                            nc.gpsimd.tensor_tensor(
                                maccs[mj][:], maccs[mj][:], pr[:], AluOp.add,
                            )
                    elif te == TO - 1:
                        cur = r1s.pop(te)
                        for mj in range(EC // 128):
                            nc.gpsimd.tensor_tensor(
                                maccs[mj][:], maccs[mj][:], cur[mj][:],
                                AluOp.add,
                            )

                if s < TO:
                    hg = work.tile([64, N], F16, tag="hg", bufs=2)
                    nc.vector.tensor_tensor(hg[:], tnh[:], sgm[:], AluOp.mult)
                    hgs[s] = hg

            # ---- end2 on the time-sum (scale 1/TO + bias fused below) ----
            e2ps = ps.tile([P, N], FP, tag="pa", bufs=1)
            for kj in range(EC // 128):
                nc.tensor.matmul(
                    e2ps[:],
                    e2t[:, kj * P: (kj + 1) * P],
                    maccs[kj][:],
                    start=(kj == 0),
                    stop=(kj == EC // 128 - 1),
                )
            outsb = consts.tile([P, N], FP)
            nc.scalar.activation(
                outsb[:], e2ps[:], Act.Identity,
                bias=cf[0:P, o_e2b: o_e2b + 1], scale=1.0 / TO,
            )
            nc.sync.dma_start(out=d_out[:], in_=outsb[:])

    return nc


_NC_CACHE = {}


def _get_nc():
    if "nc" not in _NC_CACHE:
        nc = _build_nc()
        nc.finalize()
        _NC_CACHE["nc"] = nc
    return _NC_CACHE["nc"]


def kernel(x, edge_index, edge_weight, start_w, start_b, filt_w, filt_b,
           gate_w, gate_b, gcn_w, gcn_b, res_w, res_b, skip_w, skip_b,
           end1_w, end1_b, end2_w, end2_b, **_unused):
    x = np.asarray(x, dtype=np.float64)
    A = _gcn_adj(edge_index, edge_weight, N)          # float64 [tgt, src]
    rowsum = A.sum(axis=1)

    f64 = lambda a: np.asarray(a, dtype=np.float64)  # noqa: E731
    s = f64(start_w)[:, 0]
    sb = f64(start_b)
    fw, gw = f64(filt_w), f64(gate_w)
    gcn = f64(gcn_w)
    v0 = gcn @ (fw[:, :, 0] @ s)
    v1 = gcn @ (fw[:, :, 1] @ s)
    bfg = gcn @ ((fw[:, :, 0] + fw[:, :, 1]) @ sb + f64(filt_b))
    p0 = gw[:, :, 0] @ s
    p1 = gw[:, :, 1] @ s
    bgv = (gw[:, :, 0] + gw[:, :, 1]) @ sb + f64(gate_b)

    def part(a, ktiles):  # [(ktiles*128), M] -> [128, ktiles*M]
        a = np.asarray(a)
        return a.reshape(ktiles, 128, -1).transpose(1, 0, 2).reshape(128, -1)

    AT = np.ascontiguousarray(A.T)                     # [src n, tgt m]

    # per-t fg/g lhsT matrices, stacked along free dim
    wt = np.zeros((KR, TO * 128))
    for t in range(TO):
        c = t * 128
        wt[t, c:c + 64] = v0
        wt[t + 1, c:c + 64] = v1
        wt[64, c:c + 64] = bfg
        wt[65, c:c + 64] = f64(gcn_b)
        wt[32 + t, c + 64:c + 128] = p0
        wt[32 + t + 1, c + 64:c + 128] = p1
        wt[65, c + 64:c + 128] = bgv

    pack16 = np.zeros((128, _F16), dtype=np.float16)

    def put16(nm, arr):
        a = np.asarray(arr, dtype=np.float16)
        pack16[:a.shape[0], _OFF16[nm]:_OFF16[nm] + a.shape[1]] = a

    put16("at", part(AT, NT))
    put16("wt", wt)
    put16("skt", f64(skip_w).T)
    put16("e1t", part(f64(end1_w).T, SC // 128))

    pack32 = np.zeros((128, _F32), dtype=np.float32)

    def put32(nm, arr):
        a = np.asarray(arr, dtype=np.float32)
        pack32[:a.shape[0], _OFF32[nm]:_OFF32[nm] + a.shape[1]] = a

    put32("skb", f64(skip_b).reshape(SC // 128, 128).T)
    put32("e1b", f64(end1_b).reshape(EC // 128, 128).T)
    put32("e2t", part(f64(end2_w).T, EC // 128))
    put32("e2b", np.asarray(end2_b).reshape(P, 1))

    in_maps = []
    for b in range(B):
        pk = pack16.copy()
        xb = x[b]                                      # [T, N]
        pk[:, _OFF16["xT"]:_OFF16["xT"] + NT * T] = part(xb.T, NT)
        xnr = np.zeros((34, N))
        xnr[0:32] = xb
        xnr[32] = rowsum
        xnr[33] = 1.0
        pk[0:34, _OFF16["xnr"]:_OFF16["xnr"] + N] = xnr.astype(np.float16)
        in_maps.append({"H": pk, "F": pack32})

    nc = _get_nc()
    _NC_CACHE["in_maps"] = in_maps
    res = run_bass_kernel_spmd(nc, in_maps, list(range(NCORES)))
    out = np.stack([res.results[i]["out"] for i in range(B)])
    return out.astype(np.float32)                       # [B, P, N]


# revision 14
# speedup vs baseline: 2.2395x; 1.2685x over previous
"""GraphWaveNet block kernel for 8 Trainium2 NeuronCores (v2).

Math (reference reduced; res_w branch is dead code):
  A = gcn_norm adjacency [N,N]; xa[m,t] = sum_n A[m,n] x[t,n]
  fg[o,m,t] = v0[o] xa[m,t] + v1[o] xa[m,t+1] + rowsum[m] bfg[o] + gcn_b[o]
  g [o,n,t] = p0[o] x[t,n]  + p1[o] x[t+1,n]  + bg[o]
  hg = tanh(fg) * sigmoid(g)
  out = end2 @ mean_t relu(end1 @ relu(skip @ hg + skb) + e1b) / TO + e2b

Design vs v1 baseline:
  - fg AND g for one t come out of a single K=66 fp16 matmul, channel-major
    (contract over a stacked rhs R = [xaT(32); x(32); rowsum; ones]), so no
    PE transposes and no broadcast elementwise ops.
  - all big GEMMs run in fp16 (PE full rate, half the SBUF read power of
    fp32r -> HAM stays at 2.4 GHz), PSUM accumulation stays fp32.
  - loop software-skewed: fg at t, skip at t-1, end1 at t-2, so the PE
    queue never waits on Act/DVE within a step.
  - relu+bias split across Act (activation bias operand) and DVE
    (tensor_scalar add+max); time-mean via fp16 pair-adds on DVE and
    fp32 accumulation on the otherwise idle GpSimd.

Sharding: data-parallel over batch, 1 batch element per core (B=8).
"""

import numpy as np

import concourse.bass as bass  # noqa: F401  (engine types via bacc)
from concourse import bacc
from concourse import mybir
from concourse.bass_utils import run_bass_kernel_spmd
from concourse.tile import TileContext

FP = mybir.dt.float32
FPR = mybir.dt.float32r
F16 = mybir.dt.float16

B, T, N, E = 8, 32, 512, 8192
TO = T - 1          # output time steps
RC = DC = 64
SC, EC, P = 256, 512, 12
NCORES = 8
NT = N // 128       # node tiles
KR = 66             # stacked-rhs contraction: 32 xaT + 32 x + rowsum + ones

# fp16 packed-constant layout: name -> free-dim width of the [128, w] segment
_SEGS16 = [
    ("xT", NT * T),            # lhsT tiles for phase A  [128 n, (kt, t)]
    ("at", NT * N),            # A^T tiles (rhs phase A) [128 n, (kt, m)]
    ("xnr", N),                # rows 0-31 x natural; row 32 rowsum; row 33 ones
    ("wt", TO * 128),          # rows 0-65: per-t fg/g lhsT matrices
    ("skt", SC),               # rows 0-63: skip_w^T
    ("e1t", (SC // 128) * EC),  # end1_w^T tiles [128, (kj, m)]
    ("e2t", (EC // 128) * P),   # end2_w^T tiles [128, (kj, u)]
]
_OFF16 = {}
_F16 = 0
for _nm, _w in _SEGS16:
    _OFF16[_nm] = _F16
    _F16 += _w

# fp32 packed-constant layout
_SEGS32 = [
    ("skb", SC // 128),
    ("e1b", EC // 128),
    ("e2b", 1),
]
_OFF32 = {}
_F32 = 0
for _nm, _w in _SEGS32:
    _OFF32[_nm] = _F32
    _F32 += _w


def _gcn_adj(edge_index, edge_weight, n):
    ei = np.asarray(edge_index)
    ew = np.asarray(edge_weight, dtype=np.float64)
    ar = np.arange(n)
    row = np.concatenate([ei[0], ar])
    col = np.concatenate([ei[1], ar])
    w = np.concatenate([ew, np.ones(n)])
    deg = np.zeros(n)
    np.add.at(deg, col, w)
    dis = np.where(deg > 0, 1.0 / np.sqrt(np.maximum(deg, 1e-300)), 0.0)
    norm = dis[row] * w * dis[col]
    A = np.zeros((n, n))
    np.add.at(A, (col, row), norm)
    return A  # A[tgt, src]


def _build_nc():
    nc = bacc.Bacc()
    d_h = nc.declare_dram_parameter("H", [128, _F16], F16, isOutput=False)
    d_f = nc.declare_dram_parameter("F", [128, _F32], FP, isOutput=False)
    d_out = nc.declare_dram_parameter("out", [P, N], FP, isOutput=True)

    AluOp = mybir.AluOpType
    Act = mybir.ActivationFunctionType

    o_xT = _OFF16["xT"]
    o_at = _OFF16["at"]
    o_xnr = _OFF16["xnr"]
    o_wt = _OFF16["wt"]
    o_skt = _OFF16["skt"]
    o_e1t = _OFF16["e1t"]
    o_e2t = _OFF16["e2t"]
    o_skb = _OFF32["skb"]
    o_e1b = _OFF32["e1b"]
    o_e2b = _OFF32["e2b"]

    with TileContext(nc) as tc:
        with (
            tc.tile_pool(name="consts", bufs=1) as consts,
            tc.tile_pool(name="work", bufs=2) as work,
            tc.tile_pool(name="accum", bufs=1) as accum,
            tc.tile_pool(name="ps", bufs=1, space="PSUM") as ps,
        ):
            ct = consts.tile([128, _F16], F16)
            # DMA 1: phase-A inputs (xT + at); DMA 2: everything else.
            nc.sync.dma_start(out=ct[:, :o_xnr], in_=d_h[:, :o_xnr])
            nc.sync.dma_start(out=ct[:, o_xnr:], in_=d_h[:, o_xnr:])
            cf = consts.tile([128, _F32], FP)
            nc.sync.dma_start(out=cf[:], in_=d_f[:])
            # stacked rhs R: rows 0-31 xaT (computed), 32-63 x, 64 rowsum,
            # 65 ones
            R = consts.tile([KR, N], F16)
            nc.sync.dma_start(out=R[32:KR], in_=d_h[0:34, o_xnr:o_xnr + N])

            # prime the one activation table set during the DMA wait
            prime = consts.tile([1, 1], FP)
            nc.vector.memset(prime[:], 0.0)
            nc.scalar.activation(prime[:], prime[:], Act.Sigmoid)

            # ---- phase A: xaT[t, m] = sum_n x[t, n] AT[n, m] ----
            xaps = ps.tile([32, N], FP, tag="pa", bufs=1)
            for kt in range(NT):
                nc.tensor.matmul(
                    xaps[:],
                    ct[:, o_xT + kt * T: o_xT + (kt + 1) * T],
                    ct[:, o_at + kt * N: o_at + (kt + 1) * N],
                    start=(kt == 0),
                    stop=(kt == NT - 1),
                )
            nc.scalar.copy(out=R[0:32], in_=xaps[:])

            # end2 accumulator: one PSUM bank, one accumulation group
            # spanning all (t, kj) — folds the time-mean into the matmul
            e2acc = ps.tile([P, N], FP, tag="pa", bufs=1)

            hgs = {}
            rsss = {}
            r1s = {}

            # skewed main loop: fg at s, skip at s-1, end1 at s-2, end2 at s-3
            for s in range(TO + 3):
                if s < TO:
                    t = s
                    fgps = ps.tile([128, N], FP, tag="fg", bufs=2)
                    nc.tensor.matmul(
                        fgps[:],
                        ct[0:KR, o_wt + t * 128: o_wt + (t + 1) * 128],
                        R[:],
                        start=True,
                        stop=True,
                    )
                    tnh = work.tile([64, N], F16, tag="tnh", bufs=2)
                    sgm = work.tile([64, N], F16, tag="sgm", bufs=2)
                    nc.scalar.activation(tnh[:], fgps[0:64], Act.Tanh)
                    nc.scalar.activation(sgm[:], fgps[64:128], Act.Sigmoid)

                tk = s - 1
                if 0 <= tk < TO:
                    hg = hgs.pop(tk)
                    sks = []
                    for sj in range(SC // 128):
                        skps = ps.tile([128, N], FP, tag="sk", bufs=2)
                        nc.tensor.matmul(
                            skps[:],
                            ct[0:64, o_skt + sj * 128: o_skt + (sj + 1) * 128],
                            hg[:],
                            start=True,
                            stop=True,
                        )
                        sks.append(skps)
                    rss = []
                    for sj in range(SC // 128):
                        rs = work.tile([128, N], F16, tag=f"rss{sj}", bufs=3)
                        bias = cf[:, o_skb + sj: o_skb + sj + 1]
                        if sj == 0:
                            nc.scalar.activation(
                                rs[:], sks[sj][:], Act.Relu,
                                bias=bias, scale=1.0,
                            )
                        else:
                            nc.vector.tensor_scalar(
                                rs[:], sks[sj][:], bias, 0.0,
                                AluOp.add, AluOp.max,
                            )
                        rss.append(rs)
                    rsss[tk] = rss

                te = s - 2
                if 0 <= te < TO:
                    rss = rsss.pop(te)
                    e1list = []
                    for mj in range(EC // 128):
                        e1ps = ps.tile([128, N], FP, tag="e1", bufs=3)
                        for kj in range(SC // 128):
                            nc.tensor.matmul(
                                e1ps[:],
                                ct[:, o_e1t + kj * EC + mj * 128:
                                   o_e1t + kj * EC + (mj + 1) * 128],
                                rss[kj][:],
                                start=(kj == 0),
                                stop=(kj == SC // 128 - 1),
                            )
                        e1list.append(e1ps)
                    r1l = []
                    for mj in range(EC // 128):
                        r1 = work.tile([128, N], F16, tag=f"r1_{mj}", bufs=2)
                        bias = cf[:, o_e1b + mj: o_e1b + mj + 1]
                        if mj == 0:
                            nc.scalar.activation(
                                r1[:], e1list[mj][:], Act.Relu,
                                bias=bias, scale=1.0,
                            )
                        else:
                            nc.vector.tensor_scalar(
                                r1[:], e1list[mj][:], bias, 0.0,
                                AluOp.add, AluOp.max,
                            )
                        r1l.append(r1)
                    r1s[te] = r1l

                # end2 accumulation for t = s-3 (one group over all t, kj)
                t2 = s - 3
                if 0 <= t2 < TO:
                    r1l = r1s.pop(t2)
                    for kj in range(EC // 128):
                        nc.tensor.matmul(
                            e2acc[:],
                            ct[:, o_e2t + kj * P: o_e2t + (kj + 1) * P],
                            r1l[kj][:],
                            start=(t2 == 0 and kj == 0),
                            stop=(t2 == TO - 1 and kj == EC // 128 - 1),
                        )

                if s < TO:
                    hg = work.tile([64, N], F16, tag="hg", bufs=2)
                    nc.gpsimd.tensor_tensor(hg[:], tnh[:], sgm[:], AluOp.mult)
                    hgs[s] = hg

            # ---- scale 1/TO + bias on the accumulated end2 sum ----
            outsb = consts.tile([P, N], FP)
            nc.scalar.activation(
                outsb[:], e2acc[:], Act.Identity,
                bias=cf[0:P, o_e2b: o_e2b + 1], scale=1.0 / TO,
            )
            nc.sync.dma_start(out=d_out[:], in_=outsb[:])

    return nc


_NC_CACHE = {}


def _get_nc():
    if "nc" not in _NC_CACHE:
        nc = _build_nc()
        nc.finalize()
        _NC_CACHE["nc"] = nc
    return _NC_CACHE["nc"]


def kernel(x, edge_index, edge_weight, start_w, start_b, filt_w, filt_b,
           gate_w, gate_b, gcn_w, gcn_b, res_w, res_b, skip_w, skip_b,
           end1_w, end1_b, end2_w, end2_b, **_unused):
    x = np.asarray(x, dtype=np.float64)
    A = _gcn_adj(edge_index, edge_weight, N)          # float64 [tgt, src]
    rowsum = A.sum(axis=1)

    f64 = lambda a: np.asarray(a, dtype=np.float64)  # noqa: E731
    s = f64(start_w)[:, 0]
    sb = f64(start_b)
    fw, gw = f64(filt_w), f64(gate_w)
    gcn = f64(gcn_w)
    v0 = gcn @ (fw[:, :, 0] @ s)
    v1 = gcn @ (fw[:, :, 1] @ s)
    bfg = gcn @ ((fw[:, :, 0] + fw[:, :, 1]) @ sb + f64(filt_b))
    p0 = gw[:, :, 0] @ s
    p1 = gw[:, :, 1] @ s
    bgv = (gw[:, :, 0] + gw[:, :, 1]) @ sb + f64(gate_b)

    def part(a, ktiles):  # [(ktiles*128), M] -> [128, ktiles*M]
        a = np.asarray(a)
        return a.reshape(ktiles, 128, -1).transpose(1, 0, 2).reshape(128, -1)

    AT = np.ascontiguousarray(A.T)                     # [src n, tgt m]

    # per-t fg/g lhsT matrices, stacked along free dim
    wt = np.zeros((KR, TO * 128))
    for t in range(TO):
        c = t * 128
        wt[t, c:c + 64] = v0
        wt[t + 1, c:c + 64] = v1
        wt[64, c:c + 64] = bfg
        wt[65, c:c + 64] = f64(gcn_b)
        wt[32 + t, c + 64:c + 128] = p0
        wt[32 + t + 1, c + 64:c + 128] = p1
        wt[65, c + 64:c + 128] = bgv

    pack16 = np.zeros((128, _F16), dtype=np.float16)

    def put16(nm, arr):
        a = np.asarray(arr, dtype=np.float16)
        pack16[:a.shape[0], _OFF16[nm]:_OFF16[nm] + a.shape[1]] = a

    put16("at", part(AT, NT))
    put16("wt", wt)
    put16("skt", f64(skip_w).T)
    put16("e1t", part(f64(end1_w).T, SC // 128))
    put16("e2t", part(f64(end2_w).T, EC // 128))

    pack32 = np.zeros((128, _F32), dtype=np.float32)

    def put32(nm, arr):
        a = np.asarray(arr, dtype=np.float32)
        pack32[:a.shape[0], _OFF32[nm]:_OFF32[nm] + a.shape[1]] = a

    put32("skb", f64(skip_b).reshape(SC // 128, 128).T)
    put32("e1b", f64(end1_b).reshape(EC // 128, 128).T)
    put32("e2b", np.asarray(end2_b).reshape(P, 1))

    in_maps = []
    for b in range(B):
        pk = pack16.copy()
        xb = x[b]                                      # [T, N]
        pk[:, _OFF16["xT"]:_OFF16["xT"] + NT * T] = part(xb.T, NT)
        xnr = np.zeros((34, N))
        xnr[0:32] = xb
        xnr[32] = rowsum
        xnr[33] = 1.0
        pk[0:34, _OFF16["xnr"]:_OFF16["xnr"] + N] = xnr.astype(np.float16)
        in_maps.append({"H": pk, "F": pack32})

    nc = _get_nc()
    _NC_CACHE["in_maps"] = in_maps
    res = run_bass_kernel_spmd(nc, in_maps, list(range(NCORES)))
    out = np.stack([res.results[i]["out"] for i in range(B)])
    return out.astype(np.float32)                       # [B, P, N]
